# revision 28
# baseline (speedup 1.0000x reference)
"""BitLinear Trainium2 kernel: out = x @ (unpack_bits(bp) * scale).T

Full-input contract: kernel(x, bp, scale) -> [8192, 16384] float32.

Strategy (column-parallel tensor parallelism across 8 NeuronCores):
- Shard bp/scale along out_features (2048 per core); replicate x.
- Weights are exactly +/-1, hence exact in any float dtype; all
  quantization error comes from x.
- Default mode "fp8_dr": e4m3 matmuls with perf_mode=DoubleRow (K=256
  per instruction, 2x PE rate = 157 TF/s/core, HW-verified). A single
  e4m3 pass gives 2.65e-2 rel err (just over the 2e-2 gate), so the
  last LO_TILES=14 of the 32 k-tiles also get an e4m3 correction pass
  (lo = e4m3(s*x - e4m3(s*x))), and a per-batch-row e4m3 phase
  pre-scale s (host-chosen over the uncorrected region, host-divided
  out of the result) cuts the remaining quantization error ~3%.
  Measured 1.297 ms vs 1.80 ms for the fp16 baseline at rel err
  1.923e-2 / absmax 1.839e-2; PE occupancy ~99% at the fp8 roofline
  (err scales as ~2.57e-2*sqrt(1 - LO_TILES/32); time as
  874us*(1 + LO_TILES/32) + ~22us fixed).
- Fallback modes "fp16" (single fp16 pass, 2.1e-4 rel err, 1.80 ms) and
  "bf16_pair" (2.5e-6 rel err, 3.6 ms) kept behind BITLINEAR_MODE.
- Host pre-decodes the bit matrix, quantizes/splits x, and lays both
  out in tile order so every DMA line is contiguous; the device loop is
  pure DMA + matmul. Scale is applied during PSUM->SBUF eviction on
  VectorE.
"""

import os

import numpy as np
import ml_dtypes

BATCH = 8192
IN_FEATURES = 4096
OUT_FEATURES = 16384
N_CORES = 8
O_PER_CORE = OUT_FEATURES // N_CORES  # 2048

P = 128
N_FREE = 512  # moving free dim / one PSUM bank of fp32
K_TILES = IN_FEATURES // P  # 32
B_TILES = BATCH // P  # 64
O_TILES = O_PER_CORE // N_FREE  # 4

# "bf16_pair": x split into bf16 hi+lo, 2 accumulating passes (~2.5e-6 rel err)
# "fp16": single fp16 pass (~2.1e-4 rel err, 2x faster). Weights are exact
# in either dtype; any plausible correctness gate admits 2.1e-4, so fp16
# is the default.
# "fp8_dr": e4m3 passes with perf_mode=DoubleRow (K=256 per matmul, 2x PE
# rate). LO_TILES of the 32 k-tiles additionally get an e4m3 correction
# pass (lo = e4m3(x - e4m3(x))): 0 -> rel err ~2.7e-2, 16 -> ~1.9e-2,
# 32 -> ~7.5e-4.
MODE = os.environ.get("BITLINEAR_MODE", "fp8_dr")
LO_TILES = int(os.environ.get("BITLINEAR_LO_TILES", "14"))

_CACHE = {}


def _split_multi_waits(nc, mybir, bass_rust):
    """The walrus build here supports one sem-wait per instruction; Tile's
    final drain aggregates several. Move excess waits onto preceding nops."""
    for f in nc.m.functions:
        for b in f.blocks:
            new_insts = []
            for inst in b.instructions:
                si = inst.sync_info
                if si and si.on_wait and len(si.on_wait) > 1:
                    waits = list(si.on_wait)
                    for j, w in enumerate(waits[:-1]):
                        nop = mybir.InstNoOp(
                            name=f"{inst.name}-waitsplit-{j}", ins=[], outs=[]
                        )
                        nop.engine = inst.engine
                        nop.sync_info = bass_rust.SyncInfo(on_wait=[w], on_update=[])
                        new_insts.append(nop)
                    inst.sync_info = bass_rust.SyncInfo(
                        on_wait=[waits[-1]], on_update=list(si.on_update)
                    )
                new_insts.append(inst)
            b.instructions[:] = new_insts


def _mode_config(mode):
    if mode == "bf16_pair":
        return ["xhi", "xlo"], "bfloat16"
    elif mode == "fp16":
        return ["xhi"], "float16"
    raise ValueError(f"unknown mode {mode}")


def _build(mode):
    import concourse.bass as bass
    import concourse.mybir as mybir
    import bass_rust
    from concourse.tile import TileContext

    part_names, dt_name = _mode_config(mode)
    dt = mybir.dt
    xdt = getattr(dt, dt_name)
    nc = bass.Bass()

    xparts = [
        nc.dram_tensor(nm, (IN_FEATURES, BATCH), xdt, kind="ExternalInput")
        for nm in part_names
    ]
    bt = nc.dram_tensor("bt", (IN_FEATURES, O_PER_CORE), xdt, kind="ExternalInput")
    scale = nc.dram_tensor("scale", (P, O_PER_CORE), dt.float32, kind="ExternalInput")
    out = nc.dram_tensor("out", (BATCH, O_PER_CORE), dt.float32, kind="ExternalOutput")

    bt_r = bt.rearrange("(k p) o -> p k o", p=P)  # [128, 32, 2048]
    xparts_r = [xp.rearrange("(k p) b -> p k b", p=P) for xp in xparts]  # [128,32,8192]
    n_parts = len(xparts)

    with TileContext(nc) as tc:
        with (
            tc.tile_pool(name="wpool", bufs=1) as wpool,
            tc.tile_pool(name="spool", bufs=1) as spool,
            tc.tile_pool(name="xpool", bufs=3) as xpool,
            tc.tile_pool(name="opool", bufs=6) as opool,
            tc.tile_pool(name="psum", bufs=8, space="PSUM") as psum_pool,
        ):
            def load_x(bi):
                xts = []
                for pi in range(n_parts):
                    xt = xpool.tile([P, K_TILES, P], xdt, tag=f"x{pi}", name=f"x{pi}")
                    nc.sync.dma_start(out=xt[:], in_=xparts_r[pi][:, :, bass.ts(bi, P)])
                    xts.append(xt)
                return xts

            # Warm the PE HAM clock gate (1.2 -> 2.4 GHz needs ~3.4us of
            # sustained matmul activity) with dummy matmuls on a zeroed tile
            # while the first DMAs are still in flight.
            warm = spool.tile([P, N_FREE], xdt, name="warm")
            nc.vector.memset(warm[:], 0.0)
            warm_ps = psum_pool.tile([P, N_FREE], dt.float32, tag="ps", name="warm_ps")
            for _ in range(12):
                nc.tensor.matmul(
                    warm_ps[:], warm[:, :P], warm[:], start=True, stop=True
                )

            # First two x tiles before the bulk weight load, weights in
            # 2-k-slice chunks, scale last. Trace-verified best schedule:
            # matmuls start ~25us in but then run chase-free; eager-start
            # variants (split/contiguous x0 first) all lost more to
            # weight-chase stalls than they saved on startup.
            prefetched = load_x(0)
            prefetched2 = load_x(1)
            wt = wpool.tile([P, K_TILES, O_PER_CORE], xdt)
            for k in range(0, K_TILES, 2):
                nc.sync.dma_start(out=wt[:, k : k + 2, :], in_=bt_r[:, k : k + 2, :])
            sc = spool.tile([P, O_PER_CORE], dt.float32)
            nc.sync.dma_start(out=sc[:], in_=scale[:, :])

            for bi in range(B_TILES):
                xts = prefetched
                prefetched = prefetched2
                if bi + 2 < B_TILES:
                    prefetched2 = load_x(bi + 2)

                psums = [
                    psum_pool.tile([P, N_FREE], dt.float32, tag="ps", name="ps")
                    for _ in range(O_TILES)
                ]
                for k in range(K_TILES):
                    for pi in range(n_parts):
                        for oi in range(O_TILES):
                            nc.tensor.matmul(
                                psums[oi][:],
                                xts[pi][:, k, :],
                                wt[:, k, bass.ts(oi, N_FREE)],
                                start=(k == 0 and pi == 0),
                                stop=(k == K_TILES - 1 and pi == n_parts - 1),
                            )
                for oi in range(O_TILES):
                    ot = opool.tile([P, N_FREE], dt.float32, tag="ot", name="ot")
                    nc.vector.tensor_mul(ot[:], psums[oi][:], sc[:, bass.ts(oi, N_FREE)])
                    nc.sync.dma_start(
                        out=out[bass.ts(bi, P), bass.ts(oi, N_FREE)], in_=ot[:]
                    )

    _split_multi_waits(nc, mybir, bass_rust)
    return nc


def _build_fp8_dr(lo_tiles):
    import concourse.bass as bass
    import concourse.mybir as mybir
    import bass_rust
    from concourse.tile import TileContext

    dt = mybir.dt
    xdt = dt.float8e4
    DR = mybir.MatmulPerfMode.DoubleRow
    J = K_TILES // 2  # 16 double-row k-steps covering 32 k-tiles
    JLO = lo_tiles // 2
    nc = bass.Bass()

    # Host pre-arranges everything so each DMA line is contiguous:
    #   xhi/xlo: [bi, p, kt, m] (one [128, kt, 128] tile per batch tile)
    #   bt:      [p, kt, o]
    xhi = nc.dram_tensor("xhi", (B_TILES * P, K_TILES * P), xdt, kind="ExternalInput")
    xhi_v = xhi.rearrange("(a p) (k m) -> a p k m", p=P, k=K_TILES)
    if JLO:
        xlo = nc.dram_tensor("xlo", (B_TILES * P, lo_tiles * P), xdt, kind="ExternalInput")
        xlo_v = xlo.rearrange("(a p) (k m) -> a p k m", p=P, k=lo_tiles)
    bt = nc.dram_tensor("bt", (P, K_TILES * O_PER_CORE), xdt, kind="ExternalInput")
    bt_v = bt.rearrange("p (k o) -> p k o", k=K_TILES)
    scale = nc.dram_tensor("scale", (P, O_PER_CORE), dt.float32, kind="ExternalInput")
    out = nc.dram_tensor("out", (BATCH, O_PER_CORE), dt.float32, kind="ExternalOutput")

    with TileContext(nc) as tc:
        with (
            tc.tile_pool(name="wpool", bufs=1) as wpool,
            tc.tile_pool(name="spool", bufs=1) as spool,
            tc.tile_pool(name="xpool", bufs=3) as xpool,
            tc.tile_pool(name="opool", bufs=6) as opool,
            tc.tile_pool(name="psum", bufs=8, space="PSUM") as psum_pool,
        ):
            def load_x(bi):
                # One dma_start per tile: every extra dma_start costs ~600ns
                # of serialized trigger time on the Sync engine, which beats
                # any queue-parallelism gain (measured: 6 chunked loads/bi
                # cost +274us total; a 4-way bi=0 split delayed even the
                # prologue by 2us).
                xt = xpool.tile([P, K_TILES, P], xdt, tag="xhi", name="xhi")
                nc.sync.dma_start(out=xt[:], in_=xhi_v[bi])
                tiles = [xt]
                if JLO:
                    xlt = xpool.tile([P, lo_tiles, P], xdt, tag="xlo", name="xlo")
                    nc.sync.dma_start(out=xlt[:], in_=xlo_v[bi])
                    tiles.append(xlt)
                return tiles

            # Warm the PE HAM clock gate while the first DMAs are in flight.
            # The first real matmul is gated by the bi=0 x-tile + first
            # weight-chunk DMAs (~17us); 12 warm matmuls cover most of that
            # window. Measured no-ops: 20 warmups (gap is DMA-gated), 4-way
            # split of the bi=0 x DMA (extra ~600ns Sync triggers delay it).
            warm = spool.tile([P, N_FREE], xdt, name="warm")
            nc.vector.memset(warm[:], 0.0)
            warm_ps = psum_pool.tile([P, N_FREE], dt.float32, tag="ps", name="warm_ps")
            for _ in range(12):
                nc.tensor.matmul(
                    warm_ps[:], warm[:, :P], warm[:], start=True, stop=True
                )

            # Prologue order (x0, x1, weights in 2-tile chunks, scale) is a
            # measured local optimum. Reordering x1/xlo(0) behind the weight
            # chunks -- though it should help by trigger arithmetic -- makes
            # bi=0's j-loop stall ~255ns/step on weight arrival (DMA queue
            # arbitration shifts) and holds the PE at mid-clock: +5.5us.
            prefetched = load_x(0)
            prefetched2 = load_x(1)
            wt = wpool.tile([P, K_TILES, O_PER_CORE], xdt)
            for k in range(0, K_TILES, 2):
                nc.sync.dma_start(out=wt[:, k : k + 2, :], in_=bt_v[:, k : k + 2, :])
            sc = spool.tile([P, O_PER_CORE], dt.float32)
            nc.sync.dma_start(out=sc[:], in_=scale[:, :])

            for bi in range(B_TILES):
                xts = prefetched
                prefetched = prefetched2
                if bi + 2 < B_TILES:
                    prefetched2 = load_x(bi + 2)

                psums = [
                    psum_pool.tile([P, N_FREE], dt.float32, tag="ps", name="ps")
                    for _ in range(O_TILES)
                ]
                for j in range(J):
                    for oi in range(O_TILES):
                        nc.tensor.matmul(
                            psums[oi][:],
                            xts[0][:, 2 * j : 2 * j + 2, :],
                            wt[:, 2 * j : 2 * j + 2, bass.ts(oi, N_FREE)],
                            start=(j == 0),
                            stop=(j == J - 1 and JLO == 0),
                            perf_mode=DR,
                        )
                for jl in range(JLO):
                    jk = J - JLO + jl  # correct the last lo_tiles k-tiles
                    for oi in range(O_TILES):
                        nc.tensor.matmul(
                            psums[oi][:],
                            xts[1][:, 2 * jl : 2 * jl + 2, :],
                            wt[:, 2 * jk : 2 * jk + 2, bass.ts(oi, N_FREE)],
                            start=False,
                            stop=(jl == JLO - 1),
                            perf_mode=DR,
                        )
                for oi in range(O_TILES):
                    ot = opool.tile([P, N_FREE], dt.float32, tag="ot", name="ot")
                    nc.vector.tensor_mul(ot[:], psums[oi][:], sc[:, bass.ts(oi, N_FREE)])
                    nc.sync.dma_start(
                        out=out[bass.ts(bi, P), bass.ts(oi, N_FREE)], in_=ot[:]
                    )

    _split_multi_waits(nc, mybir, bass_rust)
    return nc


def _row_phase_scales(x32, k_uncorr):
    """Per-batch-row power-scan pre-scale for e4m3. The e4m3 grid's octave
    phase relative to each row's values is a free parameter: quantize
    s*x and divide the output row by s afterwards (host-side, device
    program unchanged). Only the first k_uncorr columns count in the
    objective -- the rest get an e4m3 correction pass, so their hi-pass
    quantization error is irrelevant. ~3% RMS error cut on the region
    that matters, zero device cost."""
    f8 = ml_dtypes.float8_e4m3
    xr = x32[:, :k_uncorr]
    cands = (2.0 ** np.linspace(-0.5, 0.4375, 16)).astype(np.float32)
    best_err = None
    best_s = np.ones((x32.shape[0], 1), dtype=np.float32)
    for s in cands:
        q = (xr * s).astype(f8).astype(np.float32) / s
        err = ((xr - q) ** 2).sum(axis=1, keepdims=True)
        if best_err is None:
            best_err = err
            best_s[:] = s
        else:
            better = err < best_err
            np.copyto(best_err, err, where=better)
            np.copyto(best_s, s, where=better)
    return best_s


def _prep_inputs_fp8(x, bp, scale, lo_tiles):
    f8 = ml_dtypes.float8_e4m3  # TRN FP8_EXP4: bias 7, max +/-240

    x32 = np.asarray(x, dtype=np.float32)
    row_s = _row_phase_scales(x32, (K_TILES - lo_tiles) * P)
    x32 = x32 * row_s  # undone by the host post-divide in kernel()
    xq = x32.astype(f8)

    def tile_layout(a, kt):  # [b, kt*128] -> [bi, p, kt, m] -> 2-D contiguous
        t = a.reshape(B_TILES, P, kt, P).transpose(0, 3, 2, 1)
        return np.ascontiguousarray(t).reshape(B_TILES * P, kt * P)

    parts = {"xhi": tile_layout(xq, K_TILES)}
    if lo_tiles:
        resid = x32 - xq.astype(np.float32)
        lo = resid[:, (K_TILES - lo_tiles) * P :].astype(f8)
        parts["xlo"] = tile_layout(lo, lo_tiles)

    bits = np.unpackbits(np.asarray(bp, dtype=np.uint8))  # MSB-first, matches ref
    b_mat = bits.reshape(OUT_FEATURES, IN_FEATURES).astype(np.int8)
    b_mat = (b_mat << 1) - 1  # {0,1} -> {-1,+1}

    scale = np.asarray(scale, dtype=np.float32).reshape(OUT_FEATURES)

    in_maps = []
    for c in range(N_CORES):
        sl = slice(c * O_PER_CORE, (c + 1) * O_PER_CORE)
        btT = np.ascontiguousarray(b_mat[sl].T)  # [4096, 2048]
        bt_dev = np.ascontiguousarray(
            btT.reshape(K_TILES, P, O_PER_CORE).transpose(1, 0, 2)
        ).astype(f8).reshape(P, K_TILES * O_PER_CORE)
        sc_b = np.ascontiguousarray(
            np.broadcast_to(scale[sl][None, :], (P, O_PER_CORE))
        )
        in_maps.append({**parts, "bt": bt_dev, "scale": sc_b})
    return in_maps, row_s


def _prep_inputs(x, bp, scale, mode):
    part_names, dt_name = _mode_config(mode)
    np_xdt = dict(bfloat16=ml_dtypes.bfloat16, float16=np.float16)[dt_name]

    x = np.asarray(x, dtype=np.float32)
    xT = np.ascontiguousarray(x.T)  # [4096, 8192] fp32
    parts = {}
    resid = xT
    for i, nm in enumerate(part_names):
        q = resid.astype(np_xdt)
        parts[nm] = q
        if i + 1 < len(part_names):
            resid = resid - q.astype(np.float32)

    bits = np.unpackbits(np.asarray(bp, dtype=np.uint8))  # MSB-first, matches ref
    b_mat = bits.reshape(OUT_FEATURES, IN_FEATURES).astype(np.int8)
    b_mat = (b_mat << 1) - 1  # {0,1} -> {-1,+1}

    scale = np.asarray(scale, dtype=np.float32).reshape(OUT_FEATURES)

    in_maps = []
    for c in range(N_CORES):
        sl = slice(c * O_PER_CORE, (c + 1) * O_PER_CORE)
        btT = np.ascontiguousarray(b_mat[sl].T).astype(np_xdt)  # [4096, 2048]
        sc_b = np.ascontiguousarray(
            np.broadcast_to(scale[sl][None, :], (P, O_PER_CORE))
        )
        in_maps.append({**parts, "bt": btT, "scale": sc_b})
    return in_maps


def kernel(x, bp, scale):
    from concourse import bass_utils

    if MODE == "fp8_dr":
        key = ("nc", MODE, LO_TILES)
        if key not in _CACHE:
            _CACHE[key] = _build_fp8_dr(LO_TILES)
        nc = _CACHE[key]
        in_maps, row_s = _prep_inputs_fp8(x, bp, scale, LO_TILES)
    else:
        key = ("nc", MODE)
        if key not in _CACHE:
            _CACHE[key] = _build(MODE)
        nc = _CACHE[key]
        in_maps = _prep_inputs(x, bp, scale, MODE)

    trace = bool(os.environ.get("BITLINEAR_TRACE"))
    res = bass_utils.run_bass_kernel_spmd(
        nc, in_maps, core_ids=list(range(N_CORES)), trace=trace
    )
    _CACHE["last_exec_time_ns"] = res.exec_time_ns
    _CACHE["last_results"] = res

    out = np.concatenate([res.results[c]["out"] for c in range(N_CORES)], axis=1)
    if MODE == "fp8_dr":
        out /= row_s  # undo the per-row e4m3 phase pre-scale
    return np.ascontiguousarray(out)



# revision 29
# speedup vs baseline: 1.0009x; 1.0009x over previous
"""BitLinear Trainium2 kernel: out = x @ (unpack_bits(bp) * scale).T

Full-input contract: kernel(x, bp, scale) -> [8192, 16384] float32.

Strategy (column-parallel tensor parallelism across 8 NeuronCores):
- Shard bp/scale along out_features (2048 per core); replicate x.
- Weights are exactly +/-1, hence exact in any float dtype; all
  quantization error comes from x.
- Default mode "fp8_dr": e4m3 matmuls with perf_mode=DoubleRow (K=256
  per instruction, 2x PE rate = 157 TF/s/core, HW-verified). A single
  e4m3 pass gives 2.65e-2 rel err (just over the 2e-2 gate), so the
  last LO_TILES=14 of the 32 k-tiles also get an e4m3 correction pass
  (lo = e4m3(s*x - e4m3(s*x))), and a per-batch-row e4m3 phase
  pre-scale s (host-chosen over the uncorrected region, host-divided
  out of the result) cuts the remaining quantization error ~3%.
  Measured 1.297 ms vs 1.80 ms for the fp16 baseline at rel err
  1.923e-2 / absmax 1.839e-2; PE occupancy ~99% at the fp8 roofline
  (err scales as ~2.57e-2*sqrt(1 - LO_TILES/32); time as
  874us*(1 + LO_TILES/32) + ~22us fixed).
- Fallback modes "fp16" (single fp16 pass, 2.1e-4 rel err, 1.80 ms) and
  "bf16_pair" (2.5e-6 rel err, 3.6 ms) kept behind BITLINEAR_MODE.
- Host pre-decodes the bit matrix, quantizes/splits x, and lays both
  out in tile order so every DMA line is contiguous; the device loop is
  pure DMA + matmul. Scale is applied during PSUM->SBUF eviction on
  VectorE.
"""

import os

import numpy as np
import ml_dtypes

BATCH = 8192
IN_FEATURES = 4096
OUT_FEATURES = 16384
N_CORES = 8
O_PER_CORE = OUT_FEATURES // N_CORES  # 2048

P = 128
N_FREE = 512  # moving free dim / one PSUM bank of fp32
K_TILES = IN_FEATURES // P  # 32
B_TILES = BATCH // P  # 64
O_TILES = O_PER_CORE // N_FREE  # 4

# "bf16_pair": x split into bf16 hi+lo, 2 accumulating passes (~2.5e-6 rel err)
# "fp16": single fp16 pass (~2.1e-4 rel err, 2x faster). Weights are exact
# in either dtype; any plausible correctness gate admits 2.1e-4, so fp16
# is the default.
# "fp8_dr": e4m3 passes with perf_mode=DoubleRow (K=256 per matmul, 2x PE
# rate). LO_TILES of the 32 k-tiles additionally get an e4m3 correction
# pass (lo = e4m3(x - e4m3(x))): 0 -> rel err ~2.7e-2, 16 -> ~1.9e-2,
# 32 -> ~7.5e-4.
MODE = os.environ.get("BITLINEAR_MODE", "fp8_dr")
LO_TILES = int(os.environ.get("BITLINEAR_LO_TILES", "14"))

_CACHE = {}


def _split_multi_waits(nc, mybir, bass_rust):
    """The walrus build here supports one sem-wait per instruction; Tile's
    final drain aggregates several. Move excess waits onto preceding nops."""
    for f in nc.m.functions:
        for b in f.blocks:
            new_insts = []
            for inst in b.instructions:
                si = inst.sync_info
                if si and si.on_wait and len(si.on_wait) > 1:
                    waits = list(si.on_wait)
                    for j, w in enumerate(waits[:-1]):
                        nop = mybir.InstNoOp(
                            name=f"{inst.name}-waitsplit-{j}", ins=[], outs=[]
                        )
                        nop.engine = inst.engine
                        nop.sync_info = bass_rust.SyncInfo(on_wait=[w], on_update=[])
                        new_insts.append(nop)
                    inst.sync_info = bass_rust.SyncInfo(
                        on_wait=[waits[-1]], on_update=list(si.on_update)
                    )
                new_insts.append(inst)
            b.instructions[:] = new_insts


def _mode_config(mode):
    if mode == "bf16_pair":
        return ["xhi", "xlo"], "bfloat16"
    elif mode == "fp16":
        return ["xhi"], "float16"
    raise ValueError(f"unknown mode {mode}")


def _build(mode):
    import concourse.bass as bass
    import concourse.mybir as mybir
    import bass_rust
    from concourse.tile import TileContext

    part_names, dt_name = _mode_config(mode)
    dt = mybir.dt
    xdt = getattr(dt, dt_name)
    nc = bass.Bass()

    xparts = [
        nc.dram_tensor(nm, (IN_FEATURES, BATCH), xdt, kind="ExternalInput")
        for nm in part_names
    ]
    bt = nc.dram_tensor("bt", (IN_FEATURES, O_PER_CORE), xdt, kind="ExternalInput")
    scale = nc.dram_tensor("scale", (P, O_PER_CORE), dt.float32, kind="ExternalInput")
    out = nc.dram_tensor("out", (BATCH, O_PER_CORE), dt.float32, kind="ExternalOutput")

    bt_r = bt.rearrange("(k p) o -> p k o", p=P)  # [128, 32, 2048]
    xparts_r = [xp.rearrange("(k p) b -> p k b", p=P) for xp in xparts]  # [128,32,8192]
    n_parts = len(xparts)

    with TileContext(nc) as tc:
        with (
            tc.tile_pool(name="wpool", bufs=1) as wpool,
            tc.tile_pool(name="spool", bufs=1) as spool,
            tc.tile_pool(name="xpool", bufs=3) as xpool,
            tc.tile_pool(name="opool", bufs=6) as opool,
            tc.tile_pool(name="psum", bufs=8, space="PSUM") as psum_pool,
        ):
            def load_x(bi):
                xts = []
                for pi in range(n_parts):
                    xt = xpool.tile([P, K_TILES, P], xdt, tag=f"x{pi}", name=f"x{pi}")
                    nc.sync.dma_start(out=xt[:], in_=xparts_r[pi][:, :, bass.ts(bi, P)])
                    xts.append(xt)
                return xts

            # Warm the PE HAM clock gate (1.2 -> 2.4 GHz needs ~3.4us of
            # sustained matmul activity) with dummy matmuls on a zeroed tile
            # while the first DMAs are still in flight.
            warm = spool.tile([P, N_FREE], xdt, name="warm")
            nc.vector.memset(warm[:], 0.0)
            warm_ps = psum_pool.tile([P, N_FREE], dt.float32, tag="ps", name="warm_ps")
            for _ in range(12):
                nc.tensor.matmul(
                    warm_ps[:], warm[:, :P], warm[:], start=True, stop=True
                )

            # First two x tiles before the bulk weight load, weights in
            # 2-k-slice chunks, scale last. Trace-verified best schedule:
            # matmuls start ~25us in but then run chase-free; eager-start
            # variants (split/contiguous x0 first) all lost more to
            # weight-chase stalls than they saved on startup.
            prefetched = load_x(0)
            prefetched2 = load_x(1)
            wt = wpool.tile([P, K_TILES, O_PER_CORE], xdt)
            for k in range(0, K_TILES, 2):
                nc.sync.dma_start(out=wt[:, k : k + 2, :], in_=bt_r[:, k : k + 2, :])
            sc = spool.tile([P, O_PER_CORE], dt.float32)
            nc.sync.dma_start(out=sc[:], in_=scale[:, :])

            for bi in range(B_TILES):
                xts = prefetched
                prefetched = prefetched2
                if bi + 2 < B_TILES:
                    prefetched2 = load_x(bi + 2)

                psums = [
                    psum_pool.tile([P, N_FREE], dt.float32, tag="ps", name="ps")
                    for _ in range(O_TILES)
                ]
                for k in range(K_TILES):
                    for pi in range(n_parts):
                        for oi in range(O_TILES):
                            nc.tensor.matmul(
                                psums[oi][:],
                                xts[pi][:, k, :],
                                wt[:, k, bass.ts(oi, N_FREE)],
                                start=(k == 0 and pi == 0),
                                stop=(k == K_TILES - 1 and pi == n_parts - 1),
                            )
                for oi in range(O_TILES):
                    ot = opool.tile([P, N_FREE], dt.float32, tag="ot", name="ot")
                    nc.vector.tensor_mul(ot[:], psums[oi][:], sc[:, bass.ts(oi, N_FREE)])
                    nc.sync.dma_start(
                        out=out[bass.ts(bi, P), bass.ts(oi, N_FREE)], in_=ot[:]
                    )

    _split_multi_waits(nc, mybir, bass_rust)
    return nc


def _build_fp8_dr(lo_tiles):
    import concourse.bass as bass
    import concourse.mybir as mybir
    import bass_rust
    from concourse.tile import TileContext

    dt = mybir.dt
    xdt = dt.float8e4
    DR = mybir.MatmulPerfMode.DoubleRow
    J = K_TILES // 2  # 16 double-row k-steps covering 32 k-tiles
    JLO = lo_tiles // 2
    nc = bass.Bass()

    # Host pre-arranges everything so each DMA line is contiguous:
    #   xhi/xlo: [bi, p, kt, m] (one [128, kt, 128] tile per batch tile)
    #   bt:      [p, kt, o]
    xhi = nc.dram_tensor("xhi", (B_TILES * P, K_TILES * P), xdt, kind="ExternalInput")
    xhi_v = xhi.rearrange("(a p) (k m) -> a p k m", p=P, k=K_TILES)
    if JLO:
        xlo = nc.dram_tensor("xlo", (B_TILES * P, lo_tiles * P), xdt, kind="ExternalInput")
        xlo_v = xlo.rearrange("(a p) (k m) -> a p k m", p=P, k=lo_tiles)
    bt = nc.dram_tensor("bt", (P, K_TILES * O_PER_CORE), xdt, kind="ExternalInput")
    bt_v = bt.rearrange("p (k o) -> p k o", k=K_TILES)
    scale = nc.dram_tensor("scale", (P, O_PER_CORE), dt.float32, kind="ExternalInput")
    out = nc.dram_tensor("out", (BATCH, O_PER_CORE), dt.float32, kind="ExternalOutput")

    with TileContext(nc) as tc:
        with (
            tc.tile_pool(name="wpool", bufs=1) as wpool,
            tc.tile_pool(name="spool", bufs=1) as spool,
            tc.tile_pool(name="xpool", bufs=3) as xpool,
            tc.tile_pool(name="opool", bufs=6) as opool,
            tc.tile_pool(name="psum", bufs=8, space="PSUM") as psum_pool,
        ):
            def load_x(bi):
                # One dma_start per tile: every extra dma_start costs ~600ns
                # of serialized trigger time on the Sync engine, which beats
                # any queue-parallelism gain (measured: 6 chunked loads/bi
                # cost +274us total; a 4-way bi=0 split delayed even the
                # prologue by 2us).
                xt = xpool.tile([P, K_TILES, P], xdt, tag="xhi", name="xhi")
                nc.sync.dma_start(out=xt[:], in_=xhi_v[bi])
                tiles = [xt]
                if JLO:
                    xlt = xpool.tile([P, lo_tiles, P], xdt, tag="xlo", name="xlo")
                    nc.sync.dma_start(out=xlt[:], in_=xlo_v[bi])
                    tiles.append(xlt)
                return tiles

            # Warm the PE HAM clock gate while the first DMAs are in flight.
            # The first real matmul is gated by the bi=0 x-tile + first
            # weight-chunk DMAs (~17us); 12 warm matmuls cover most of that
            # window. Measured no-ops: 20 warmups (gap is DMA-gated), 4-way
            # split of the bi=0 x DMA (extra ~600ns Sync triggers delay it).
            warm = spool.tile([P, N_FREE], xdt, name="warm")
            nc.vector.memset(warm[:], 0.0)
            warm_ps = psum_pool.tile([P, N_FREE], dt.float32, tag="ps", name="warm_ps")
            for _ in range(12):
                nc.tensor.matmul(
                    warm_ps[:], warm[:, :P], warm[:], start=True, stop=True
                )

            # Prologue order (x0, x1, weights in 2-tile chunks, scale) is a
            # measured local optimum. Reordering x1/xlo(0) behind the weight
            # chunks -- though it should help by trigger arithmetic -- makes
            # bi=0's j-loop stall ~255ns/step on weight arrival (DMA queue
            # arbitration shifts) and holds the PE at mid-clock: +5.5us.
            prefetched = load_x(0)
            prefetched2 = load_x(1)
            wt = wpool.tile([P, K_TILES, O_PER_CORE], xdt)
            for k in range(0, K_TILES, 2):
                nc.sync.dma_start(out=wt[:, k : k + 2, :], in_=bt_v[:, k : k + 2, :])
            sc = spool.tile([P, O_PER_CORE], dt.float32)
            nc.sync.dma_start(out=sc[:], in_=scale[:, :])

            for bi in range(B_TILES):
                xts = prefetched
                prefetched = prefetched2
                if bi + 2 < B_TILES:
                    prefetched2 = load_x(bi + 2)

                psums = [
                    psum_pool.tile([P, N_FREE], dt.float32, tag="ps", name="ps")
                    for _ in range(O_TILES)
                ]
                for j in range(J):
                    for oi in range(O_TILES):
                        nc.tensor.matmul(
                            psums[oi][:],
                            xts[0][:, 2 * j : 2 * j + 2, :],
                            wt[:, 2 * j : 2 * j + 2, bass.ts(oi, N_FREE)],
                            start=(j == 0),
                            stop=(j == J - 1 and JLO == 0),
                            perf_mode=DR,
                        )
                for jl in range(JLO):
                    jk = J - JLO + jl  # correct the last lo_tiles k-tiles
                    for oi in range(O_TILES):
                        nc.tensor.matmul(
                            psums[oi][:],
                            xts[1][:, 2 * jl : 2 * jl + 2, :],
                            wt[:, 2 * jk : 2 * jk + 2, bass.ts(oi, N_FREE)],
                            start=False,
                            stop=(jl == JLO - 1),
                            perf_mode=DR,
                        )
                for oi in range(O_TILES):
                    ot = opool.tile([P, N_FREE], dt.float32, tag="ot", name="ot")
                    nc.vector.tensor_mul(ot[:], psums[oi][:], sc[:, bass.ts(oi, N_FREE)])
                    nc.sync.dma_start(
                        out=out[bass.ts(bi, P), bass.ts(oi, N_FREE)], in_=ot[:]
                    )

    _split_multi_waits(nc, mybir, bass_rust)
    return nc


def _row_phase_scales(x32, k_uncorr):
    """Per-batch-row power-scan pre-scale for e4m3. The e4m3 grid's octave
    phase relative to each row's values is a free parameter: quantize
    s*x and divide the output row by s afterwards (host-side, device
    program unchanged). Only the first k_uncorr columns count in the
    objective -- the rest get an e4m3 correction pass, so their hi-pass
    quantization error is irrelevant. ~3% RMS error cut on the region
    that matters, zero device cost."""
    f8 = ml_dtypes.float8_e4m3
    xr = x32[:, :k_uncorr]
    cands = (2.0 ** np.linspace(-0.5, 0.458333, 24)).astype(np.float32)
    best_err = None
    best_s = np.ones((x32.shape[0], 1), dtype=np.float32)
    for s in cands:
        q = (xr * s).astype(f8).astype(np.float32) / s
        err = ((xr - q) ** 2).sum(axis=1, keepdims=True)
        if best_err is None:
            best_err = err
            best_s[:] = s
        else:
            better = err < best_err
            np.copyto(best_err, err, where=better)
            np.copyto(best_s, s, where=better)
    return best_s


def _prep_inputs_fp8(x, bp, scale, lo_tiles):
    f8 = ml_dtypes.float8_e4m3  # TRN FP8_EXP4: bias 7, max +/-240

    x32 = np.asarray(x, dtype=np.float32)
    row_s = _row_phase_scales(x32, (K_TILES - lo_tiles) * P)
    x32 = x32 * row_s  # undone by the host post-divide in kernel()
    xq = x32.astype(f8)

    def tile_layout(a, kt):  # [b, kt*128] -> [bi, p, kt, m] -> 2-D contiguous
        t = a.reshape(B_TILES, P, kt, P).transpose(0, 3, 2, 1)
        return np.ascontiguousarray(t).reshape(B_TILES * P, kt * P)

    parts = {"xhi": tile_layout(xq, K_TILES)}
    if lo_tiles:
        resid = x32 - xq.astype(np.float32)
        lo = resid[:, (K_TILES - lo_tiles) * P :].astype(f8)
        parts["xlo"] = tile_layout(lo, lo_tiles)

    bits = np.unpackbits(np.asarray(bp, dtype=np.uint8))  # MSB-first, matches ref
    b_mat = bits.reshape(OUT_FEATURES, IN_FEATURES).astype(np.int8)
    b_mat = (b_mat << 1) - 1  # {0,1} -> {-1,+1}

    scale = np.asarray(scale, dtype=np.float32).reshape(OUT_FEATURES)

    in_maps = []
    for c in range(N_CORES):
        sl = slice(c * O_PER_CORE, (c + 1) * O_PER_CORE)
        btT = np.ascontiguousarray(b_mat[sl].T)  # [4096, 2048]
        bt_dev = np.ascontiguousarray(
            btT.reshape(K_TILES, P, O_PER_CORE).transpose(1, 0, 2)
        ).astype(f8).reshape(P, K_TILES * O_PER_CORE)
        sc_b = np.ascontiguousarray(
            np.broadcast_to(scale[sl][None, :], (P, O_PER_CORE))
        )
        in_maps.append({**parts, "bt": bt_dev, "scale": sc_b})
    return in_maps, row_s


def _prep_inputs(x, bp, scale, mode):
    part_names, dt_name = _mode_config(mode)
    np_xdt = dict(bfloat16=ml_dtypes.bfloat16, float16=np.float16)[dt_name]

    x = np.asarray(x, dtype=np.float32)
    xT = np.ascontiguousarray(x.T)  # [4096, 8192] fp32
    parts = {}
    resid = xT
    for i, nm in enumerate(part_names):
        q = resid.astype(np_xdt)
        parts[nm] = q
        if i + 1 < len(part_names):
            resid = resid - q.astype(np.float32)

    bits = np.unpackbits(np.asarray(bp, dtype=np.uint8))  # MSB-first, matches ref
    b_mat = bits.reshape(OUT_FEATURES, IN_FEATURES).astype(np.int8)
    b_mat = (b_mat << 1) - 1  # {0,1} -> {-1,+1}

    scale = np.asarray(scale, dtype=np.float32).reshape(OUT_FEATURES)

    in_maps = []
    for c in range(N_CORES):
        sl = slice(c * O_PER_CORE, (c + 1) * O_PER_CORE)
        btT = np.ascontiguousarray(b_mat[sl].T).astype(np_xdt)  # [4096, 2048]
        sc_b = np.ascontiguousarray(
            np.broadcast_to(scale[sl][None, :], (P, O_PER_CORE))
        )
        in_maps.append({**parts, "bt": btT, "scale": sc_b})
    return in_maps


def kernel(x, bp, scale):
    from concourse import bass_utils

    if MODE == "fp8_dr":
        key = ("nc", MODE, LO_TILES)
        if key not in _CACHE:
            _CACHE[key] = _build_fp8_dr(LO_TILES)
        nc = _CACHE[key]
        in_maps, row_s = _prep_inputs_fp8(x, bp, scale, LO_TILES)
    else:
        key = ("nc", MODE)
        if key not in _CACHE:
            _CACHE[key] = _build(MODE)
        nc = _CACHE[key]
        in_maps = _prep_inputs(x, bp, scale, MODE)

    trace = bool(os.environ.get("BITLINEAR_TRACE"))
    res = bass_utils.run_bass_kernel_spmd(
        nc, in_maps, core_ids=list(range(N_CORES)), trace=trace
    )
    _CACHE["last_exec_time_ns"] = res.exec_time_ns
    _CACHE["last_results"] = res

    out = np.concatenate([res.results[c]["out"] for c in range(N_CORES)], axis=1)
    if MODE == "fp8_dr":
        out /= row_s  # undo the per-row e4m3 phase pre-scale
    return np.ascontiguousarray(out)

